# revision 6
# baseline (speedup 1.0000x reference)
"""BBoxDetector (PointNet++-style) on 8 Trainium2 NeuronCores.

Sharding: pure data parallelism — batch dim B=8 across the 8 cores
(1 point cloud per core) via one pmap-compiled program.

Structure:
  * FPS (1024-step sequential argmax chain over 1024 instance points) is
    inherently latency-bound; it runs vectorized over all 8 clouds on the
    host with the reference's exact fp32 update formula. Only its [B,1024,3]
    ordered-anchor output feeds the device program.
  * Everything heavy (ball queries via distance matmuls + cumsum +
    binary-search first-k extraction, neighborhood gathers, the four
    shared-MLP set-abstraction stages, FC head) runs on-device, one cloud
    per NeuronCore.

Algorithmic notes:
  * FPS is prefix-stable: FPS(P,k) of an FPS-ordered set is its first k
    points. sa1 orders ALL 1024 anchors, so sa2/sa3's FPS are prefixes
    xyz1[:256] / xyz1[:64] — one FPS instead of three.
  * neuronx-cc rejects XLA variadic reduces (argmax/argmin) and scatter is
    risky; ball-query "first nsample in index order within radius" is
    expressed as: inball mask -> cumsum -> branchless binary search for the
    (k+1)-th set bit. This exactly reproduces the reference's
    sort-then-truncate semantics including empty/overfull ball padding.
"""

import numpy as np
import jax
import jax.numpy as jnp

EPS = 1e-5
B, N, M = 8, 8192, 1024


# ----------------------------------------------------------------------
# Host-side FPS (vectorized over the batch; reference-exact fp32 updates)
# ----------------------------------------------------------------------
def _fps_order_host(inst):
    """inst [B,M,3] float32 -> ordered anchors [B,M,3] (FPS order).

    Same fp32 op order as the reference ((x-c)^2 summed x,y then z), so the
    argmax selections match bit-for-bit.
    """
    b, m, _ = inst.shape
    X = np.ascontiguousarray(inst[:, :, 0])
    Y = np.ascontiguousarray(inst[:, :, 1])
    Z = np.ascontiguousarray(inst[:, :, 2])
    dist = np.full((b, m), 1e10, np.float32)
    far = np.zeros((b,), np.int64)
    order = np.empty((b, m), np.int64)
    ar = np.arange(b)
    t = np.empty((b, m), np.float32)
    d = np.empty((b, m), np.float32)
    for k in range(m):
        order[:, k] = far
        np.subtract(X, X[ar, far][:, None], out=t)
        np.multiply(t, t, out=d)
        np.subtract(Y, Y[ar, far][:, None], out=t)
        np.multiply(t, t, out=t)
        np.add(d, t, out=d)
        np.subtract(Z, Z[ar, far][:, None], out=t)
        np.multiply(t, t, out=t)
        np.add(d, t, out=d)
        np.minimum(dist, d, out=dist)
        far = dist.argmax(1)
    return inst[ar[:, None], order]  # [B,M,3]


# ----------------------------------------------------------------------
# Device-side network (per cloud)
# ----------------------------------------------------------------------
def _first_k_indices(csum, nsample, n):
    """csum [S,N] nondecreasing fp32 counts; returns [S,nsample] int32 —
    index of the (k+1)-th set bit, or n if it doesn't exist."""
    S = csum.shape[0]
    t = jnp.arange(1, nsample + 1, dtype=jnp.float32)[None, :]  # [1,K]
    t = jnp.broadcast_to(t, (S, nsample))
    pos = jnp.zeros((S, nsample), jnp.int32)
    step = n // 2
    while step >= 1:
        probe = pos + (step - 1)
        v = jnp.take_along_axis(csum, probe, axis=1)
        pos = jnp.where(v < t, pos + step, pos)
        step //= 2
    # pos in [0, n-1]; if even csum[pos] < t the answer is n (no such bit)
    v = jnp.take_along_axis(csum, pos, axis=1)
    pos = jnp.where(v < t, n, pos)
    return pos


def _ball_group(radius, nsample, xyz, new_xyz, feats=None):
    """xyz [n,3] source points, new_xyz [S,3] centers.
    Returns grouped [S,nsample,3(+C)] = (gathered xyz - center [, feats])."""
    n = xyz.shape[0]
    # Q[s,j] = |p_j|^2 - 2 c_s . p_j  ;  inball  <=>  Q <= r^2 - |c_s|^2
    p2 = jnp.sum(xyz * xyz, -1)  # [n]
    q = p2[None, :] - 2.0 * (new_xyz @ xyz.T)  # [S,n]
    thr = radius * radius - jnp.sum(new_xyz * new_xyz, -1)  # [S]
    inball = (q <= thr[:, None]).astype(jnp.float32)  # [S,n]
    csum = jnp.cumsum(inball, axis=-1)  # [S,n]
    pos = _first_k_indices(csum, nsample, n)  # [S,K], n marks empty
    first = pos[:, :1]
    pos = jnp.where(pos == n, first, pos)
    idx = jnp.minimum(pos, n - 1)  # [S,K] int32
    g = jnp.take(xyz, idx.reshape(-1), axis=0).reshape(*idx.shape, 3)
    g = g - new_xyz[:, None, :]
    if feats is not None:
        f = jnp.take(feats, idx.reshape(-1), axis=0).reshape(
            *idx.shape, feats.shape[-1]
        )
        g = jnp.concatenate([g, f], -1)
    return g


def _mlp_max(x, layers):
    # x [S,K,C]; shared 1x1-conv MLP (BN eval mode) then max over K
    s = 1.0 / np.sqrt(1.0 + EPS)
    for w, b, g, beta in layers:
        x = jnp.einsum("skc,oc->sko", x, w) + b
        x = g * (x * s) + beta
        x = jax.nn.relu(x)
    return jnp.max(x, axis=1)  # [S,C_out]


def _rest_forward(scene, xyz1, params):
    """scene [N,3]; xyz1 [M,3] FPS-ordered anchors."""
    g1 = _ball_group(0.1, 32, scene, xyz1)  # [1024,32,3]
    p1 = _mlp_max(g1, params["sa1"])  # [1024,128]

    xyz2 = xyz1[:256]
    g2 = _ball_group(0.2, 48, xyz1, xyz2, p1)  # [256,48,131]
    p2 = _mlp_max(g2, params["sa2"])  # [256,256]

    xyz3 = xyz1[:64]
    g3 = _ball_group(0.4, 64, xyz2, xyz3, p2)  # [64,64,259]
    p3 = _mlp_max(g3, params["sa3"])  # [64,512]

    g4 = jnp.concatenate([xyz3, p3], -1)[None]  # [1,64,515]
    p4 = _mlp_max(g4, params["sa4"])  # [1,1024]

    x = p4.reshape(1024)
    s = 1.0 / np.sqrt(1.0 + EPS)
    for key in ("fc1", "fc2"):
        w, b, g, beta = params[key]
        x = x @ w.T + b
        x = jax.nn.relu(g * (x * s) + beta)
    w3, b3 = params["fc3"]
    x = x @ w3.T + b3
    return x.reshape(8, 3)


_PFN = None
_PCACHE = {"fp": None, "dev_params": None}


def _get_pfn():
    global _PFN
    if _PFN is None:
        # params are a mapped (pre-replicated, device-resident) axis so they
        # are not re-shipped over the axon tunnel on every call.
        _PFN = jax.pmap(_rest_forward, in_axes=(0, 0, 0))
    return _PFN


def _fingerprint(params):
    leaves = jax.tree.leaves(params)
    return tuple(
        (l.shape, str(l.dtype), float(np.asarray(l).sum())) for l in leaves
    )


def _device_params(params):
    fp = _fingerprint(params)
    if _PCACHE["fp"] != fp:
        devs = jax.devices()[:B]
        stacked = jax.tree.map(
            lambda l: np.broadcast_to(np.asarray(l), (B,) + np.shape(l)), params
        )
        shard = jax.tree.map(
            lambda l: jax.device_put_sharded([l[i] for i in range(B)], devs),
            stacked,
        )
        _PCACHE["fp"] = fp
        _PCACHE["dev_params"] = shard
    return _PCACHE["dev_params"]


def kernel(scene_points, instance_points, params):
    scene_points = np.ascontiguousarray(np.asarray(scene_points), np.float32)
    instance_points = np.ascontiguousarray(
        np.asarray(instance_points), np.float32
    )
    params = jax.tree.map(np.asarray, params)

    pfn = _get_pfn()
    dev_params = _device_params(params)
    xyz1 = _fps_order_host(instance_points)  # [B,M,3]
    out = pfn(scene_points, xyz1, dev_params)  # [B,8,3]
    return np.asarray(out).astype(np.float32)


if __name__ == "__main__":
    import reference

    inputs = reference.setup_inputs()
    cpu = jax.devices("cpu")[0]
    cin = jax.tree.map(lambda x: jax.device_put(np.asarray(x), cpu), inputs)
    with jax.default_device(cpu):
        exp = np.asarray(reference.reference(**cin))
    act = kernel(**jax.tree.map(np.asarray, inputs))
    print("Relative error:", np.abs(act - exp).max() / np.abs(exp).max())
